# revision 8
# baseline (speedup 1.0000x reference)
"""MLA (DeepSeek-style multi-head latent attention) distributed Bass kernel
for 8 TRN2 NeuronCores — v3 (schedule restructure).

Problem shapes (hardcoded):
  x (2, 2048, 2048), DIM=2048, N_HEADS=16, Q_LORA=1536, KV_LORA=512,
  QK_NOPE=128, QK_ROPE=64, V_HEAD=128, causal SDPA, scale=192**-0.5.

v3 changes vs v2 (which ran 564us with ~50us of Tensor stalls):
  * warmup collective dispatched first so the one-time CC barrier/rendezvous
    overlaps phase-1 compute instead of delaying the kv AllGather.
  * ONE merged q AllToAll (nope for all 16 heads + rope pairs, 1.5MB/core
    eff) replacing two serialized A2As with an 18us CC-stream hole.
  * queue hygiene: scalar queue carries ACT compute only; all AG-gated
    gathered-kv loads live on the gpsimd queue (naturally ordered after the
    AG dispatch), so no head-of-line blocking of phase-1 copies; wo
    prefetch issues on sync BEFORE the (A2A-gated) q receives.
  * phase-2 k/v production sits between q production and attention in the
    Tensor stream, covering the q A2A flight; attention starts as soon as
    the receives land.

phase 1 (token-parallel, 512 tokens/core):
  kv branch first: kv latent + roped k_pe + inv-rms row -> AllGather
  (577 rows) hidden under the q branch.
  q branch: q_lora -> rmsnorm locally -> full wq_b projection for ALL
  16 heads + rope -> merged AllToAll shipping each peer's 2 heads
  (nope) + its rope pair.
phase 2 (head-parallel, 2 heads/core): k/v production from the gathered
  latent (covers the q A2A flight), then causal flash attention (S^T
  formulation, exp without max-subtraction).
phase 3: wo partials; even half interleaved after h1 attention during the
  second out-A2A flight; odd half as the tail. AllToAll per local head
  ships UNNORMALIZED attention outputs + softmax denominators;
  normalization happens receiver-side.

All matmul operands are bfloat16 (fp32 PSUM accumulation); fp32 for the
rms/softmax statistics chains. Activations are feature-major
[features(partitions), tokens(free)] -- no transposes anywhere.
"""
import sys

sys.path.insert(0, "/opt/trn_rl_repo")

import numpy as np
import ml_dtypes

import concourse.bacc as bacc
import concourse.mybir as mybir
import concourse.tile as tile
from concourse.bass_utils import run_bass_kernel_spmd

BF = ml_dtypes.bfloat16
F32 = mybir.dt.float32
CD = mybir.dt.bfloat16
AFT = mybir.ActivationFunctionType

DIM = 2048
H = 16
QL = 1536
KVL = 512
DN = 128          # qk_nope
DR = 64           # qk_rope
DV = 128          # v head dim
B, S = 2, 2048
T = B * S
NC = 8
TPC = T // NC     # 512 tokens per core
HPC = H // NC     # 2 heads per core
SCALE = (DN + DR) ** -0.5
EPS = 1e-6
P = 128
CHUNK = 512
NCH = T // CHUNK
KVE = KVL + 2 * DR   # 640 phase-1 kv output cols (incl swapped-rope block)
KVG = KVL + DR + 1   # 577 gathered kv rows (+ inv_rms row)
NM_Q = H + H // 2    # 24 q-production m tiles (16 nope + 8 rope)
QSLOT = 3 * P        # 384 rows per peer in the merged q A2A


def build_nc():
    nc = bacc.Bacc("TRN2", target_bir_lowering=False, debug=False, num_devices=NC)

    xT = nc.declare_dram_parameter("xT", [DIM, TPC], CD, isOutput=False)
    wqaT = nc.declare_dram_parameter("wqaT", [QL // P, P, DIM], CD, isOutput=False)
    wkvaT = nc.declare_dram_parameter("wkvaT", [KVE // P, P, DIM], CD, isOutput=False)
    wqbT = nc.declare_dram_parameter("wqbT", [NM_Q, P, QL], CD, isOutput=False)
    wkvbTk = nc.declare_dram_parameter("wkvbTk", [2, P, KVL], CD, isOutput=False)
    wkvbTv = nc.declare_dram_parameter("wkvbTv", [KVL, 2 * DV], CD, isOutput=False)
    woT = nc.declare_dram_parameter("woT", [DIM // P, P, DIM], CD, isOutput=False)
    cosc = nc.declare_dram_parameter("cosc", [P, TPC], CD, isOutput=False)
    sinc = nc.declare_dram_parameter("sinc", [P, TPC], CD, isOutput=False)
    perm = nc.declare_dram_parameter("perm", [P, P], CD, isOutput=False)
    outT = nc.declare_dram_parameter("outT", [DIM, TPC], F32, isOutput=True)

    warm_in = nc.dram_tensor("warm_in", [1, 1], CD)
    warm_out = nc.dram_tensor("warm_out", [NC, 1], CD, addr_space="Shared")
    agkv_in = nc.dram_tensor("agkv_in", [KVG, TPC], CD)
    agkv_out = nc.dram_tensor("agkv_out", [NC * KVG, TPC], CD, addr_space="Shared")
    a2aq_in = nc.dram_tensor("a2aq_in", [NC * QSLOT, TPC], CD)
    a2aq_out = nc.dram_tensor("a2aq_out", [NC * QSLOT, TPC], CD)
    a2a_in = [nc.dram_tensor(f"a2a_in{h}", [NC * (DV + 1), CHUNK], CD)
              for h in range(HPC)]
    a2a_out = [nc.dram_tensor(f"a2a_out{h}", [NC * (DV + 1), CHUNK], CD)
               for h in range(HPC)]
    groups = [list(range(NC))]

    with tile.TileContext(nc) as tc, \
         tc.tile_pool(name="const", bufs=1) as constp:
        # warmup collective first: absorbs the one-time CC barrier cost
        # while phase-1 runs (input is uninitialized dram -- it is a pure
        # rendezvous, values are irrelevant)
        nc.gpsimd.collective_compute(
            "AllGather", mybir.AluOpType.bypass, replica_groups=groups,
            ins=[warm_in.ap().opt()], outs=[warm_out.ap().opt()])

        ones_f = constp.tile([P, 1], F32, tag="onesf")
        nc.any.memset(ones_f[:], 1.0)
        ones_col = constp.tile([P, 1], CD, tag="onesc")
        nc.vector.tensor_copy(ones_col[:], ones_f[:])
        onesr_f = constp.tile([1, P], F32, tag="onesrf")
        nc.any.memset(onesr_f[:], 1.0)
        ones_row = constp.tile([1, P], CD, tag="onesr")
        nc.vector.tensor_copy(ones_row[:], onesr_f[:])
        one_f11 = constp.tile([1, 1], F32, tag="onef11")
        nc.any.memset(one_f11[:], 1.0)
        one_1x1 = constp.tile([1, 1], CD, tag="one11")
        nc.vector.tensor_copy(one_1x1[:], one_f11[:])
        eps_t = constp.tile([1, 1], F32, tag="eps")
        nc.any.memset(eps_t[:], EPS)
        # touch Exp once now so the ACT table load happens during phase 1,
        # not at the first attention softmax
        warm_e = constp.tile([1, 1], F32, tag="warme")
        nc.scalar.activation(warm_e[:], eps_t[:], AFT.Exp, bias=0.0, scale=1.0)

        wo_sb = [None] * (DIM // P)

        # persistent phase-2 tensors
        _qp2cm = tc.tile_pool(name="p2q", bufs=1)
        qp2 = _qp2cm.__enter__()
        _kp2cm = tc.tile_pool(name="p2k", bufs=1)
        kp2 = _kp2cm.__enter__()
        _vp2cm = tc.tile_pool(name="p2v", bufs=T // P)
        vp2 = _vp2cm.__enter__()
        q_n = [qp2.tile([P, T], CD, tag=f"qn{h}", name=f"qn{h}") for h in range(HPC)]
        q_rope = [qp2.tile([2 * DR, S], CD, tag=f"qrope{h}", name=f"qrope{h}")
                  for h in range(HPC)]
        k_n = [kp2.tile([P, T], CD, tag=f"kn{h}", name=f"kn{h}") for h in range(HPC)]
        k_pe = kp2.tile([2 * DR, S], CD, tag="kpe")
        v_tok = [vp2.tile([P, HPC * DV], CD, tag="vtok", name=f"vtok{i}")
                 for i in range(T // P)]

        # phase-2 load pools (created early for stack-order discipline)
        _wp2cm = tc.tile_pool(name="p2w", bufs=1)
        wp2 = _wp2cm.__enter__()
        _akvcm = tc.tile_pool(name="p2actkv", bufs=1)
        actkvp = _akvcm.__enter__()
        _nrmcm = tc.tile_pool(name="p2nrm", bufs=2)
        nrmp = _nrmcm.__enter__()
        wkvk_sb = []
        wkvv_sb = []
        actkv_all = []
        ikv_all = []

        # ---------------- phase 1 ------------------------------------------
        _qlcm = tc.tile_pool(name="p1ql", bufs=1)
        qlp = _qlcm.__enter__()
        qn_t = [None] * (QL // P)

        with tc.tile_pool(name="p1x", bufs=1) as xp, \
             tc.tile_pool(name="p1w", bufs=3) as wp1, \
             tc.tile_pool(name="p1ps", bufs=3, space="PSUM") as ps1, \
             tc.tile_pool(name="p1ssps", bufs=2, space="PSUM") as ssps1, \
             tc.tile_pool(name="p1sq", bufs=1) as sqp, \
             tc.tile_pool(name="p1misc", bufs=2) as mp1:

            # first kv weight tile before x so the first matmul's operands
            # land as early as possible
            wkv0 = wp1.tile([P, DIM], CD, tag="w1", name="wkv0")
            nc.sync.dma_start(wkv0[:], wkvaT[0, :, :])
            xts = []
            for kb in range(DIM // P):
                xt = xp.tile([P, TPC], CD, tag=f"x{kb}")
                nc.sync.dma_start(xt[:], xT[kb * P:(kb + 1) * P, :])
                xts.append(xt)

            cosc_sb = mp1.tile([P, TPC], CD, tag="cosc")
            nc.sync.dma_start(cosc_sb[:], cosc[:, :])
            sinc_sb = mp1.tile([P, TPC], CD, tag="sinc")
            nc.sync.dma_start(sinc_sb[:], sinc[:, :])

            def lora_proj(w_param, nm, out_cb, w0=None):
                for m in range(nm):
                    if m == 0 and w0 is not None:
                        wt = w0
                    else:
                        wt = wp1.tile([P, DIM], CD, tag="w1", name=f"w1_{m}")
                        nc.sync.dma_start(wt[:], w_param[m, :, :])
                    ps = ps1.tile([P, TPC], F32, tag="ps1")
                    for kb in range(DIM // P):
                        nc.tensor.matmul(ps[:], wt[:, kb * P:(kb + 1) * P], xts[kb][:],
                                         start=(kb == 0), stop=(kb == DIM // P - 1))
                    out_cb(m, ps)

            # --- kv branch (payload shipped unnormalized + inv-rms row)
            sq_kv = []

            def kv_cb(m, ps):
                if m < 4:
                    kv = mp1.tile([P, TPC], CD, tag="pay")
                    nc.scalar.copy(kv[:], ps[:])
                    nc.sync.dma_start(agkv_in[m * P:(m + 1) * P, :], kv[:])
                    sq = sqp.tile([P, TPC], CD, tag=f"skv{m}", name=f"skv{m}")
                    nc.scalar.square(sq[:], ps[:])
                    sq_kv.append(sq)
                else:
                    # rows 0:64 deinterleaved kpe [r;i], rows 64:128 [i;r]
                    t_a = mp1.tile([DR, TPC], CD, tag="ropea")
                    nc.vector.tensor_mul(t_a[:], ps[0:DR, :], cosc_sb[0:DR, :])
                    t_b = mp1.tile([DR, TPC], CD, tag="ropeb")
                    nc.vector.tensor_mul(t_b[:], ps[DR:2 * DR, :], sinc_sb[0:DR, :])
                    kpe = mp1.tile([DR, TPC], CD, tag="ropeo")
                    nc.vector.tensor_add(kpe[:], t_a[:], t_b[:])
                    nc.sync.dma_start(agkv_in[KVL:KVL + DR, :], kpe[:])

            lora_proj(wkvaT, KVE // P, kv_cb, w0=wkv0)
            # kv inv-rms row
            ss = ssps1.tile([1, TPC], F32, tag="ss")
            for m in range(len(sq_kv)):
                nc.tensor.matmul(ss[:], ones_col[:], sq_kv[m][:],
                                 start=(m == 0), stop=(m == len(sq_kv) - 1))
            srt = mp1.tile([1, TPC], F32, tag="srt")
            nc.scalar.activation(srt[:], ss[:], AFT.Sqrt,
                                 bias=eps_t[:], scale=1.0 / KVL)
            inv_f = mp1.tile([1, TPC], F32, tag="invf")
            nc.vector.reciprocal_approx_fast(inv_f[:], srt[:])
            inv = mp1.tile([1, TPC], CD, tag="inv")
            nc.vector.tensor_copy(inv[:], inv_f[:])
            nc.sync.dma_start(agkv_in[KVL + DR:KVL + DR + 1, :], inv[:])

            nc.gpsimd.collective_compute(
                "AllGather", mybir.AluOpType.bypass, replica_groups=groups,
                ins=[agkv_in.ap().opt()], outs=[agkv_out.ap().opt()])

            # hoisted phase-2 input loads on the GPSIMD queue: they are
            # gated on the AllGather anyway and gpsimd has nothing else
            # queued, so they cause no head-of-line blocking
            for m in range(2):
                w = wp2.tile([P, KVL], CD, tag=f"wkvk{m}", name=f"wkvk{m}")
                nc.gpsimd.dma_start(w[:], wkvbTk[m, :, :])
                wkvk_sb.append(w)
            for kb in range(KVL // P):
                w = wp2.tile([P, 2 * DV], CD, tag=f"wkvv{kb}", name=f"wkvv{kb}")
                nc.gpsimd.dma_start(w[:], wkvbTv[kb * P:(kb + 1) * P, :])
                wkvv_sb.append(w)

            def load_chunk(qc, eng):
                # chunks 0-3 go on the (otherwise idle) gpsimd queue right
                # after the AG dispatch; chunks 4-7 issue later from the
                # scalar queue so their buffer-reuse waits never block the
                # merged q A2A dispatch on gpsimd
                for kb in range(KVL // P):
                    at = actkvp.tile([P, CHUNK], CD, tag="actkv",
                                     name=f"actkv{qc}_{kb}", bufs=16)
                    eng.dma_start(
                        at[:],
                        agkv_out[qc * KVG + kb * P: qc * KVG + (kb + 1) * P, :])
                    actkv_all.append(at)
                ik = actkvp.tile([1, CHUNK], CD, tag="ikv", name=f"ikv{qc}",
                                 bufs=4)
                eng.dma_start(
                    ik[:],
                    agkv_out[qc * KVG + KVL + DR: qc * KVG + KVL + DR + 1, :])
                ikv_all.append(ik)
                eng.dma_start(
                    k_pe[(qc // 4) * DR:(qc // 4 + 1) * DR,
                         (qc % 4) * CHUNK:(qc % 4 + 1) * CHUNK],
                    agkv_out[qc * KVG + KVL: qc * KVG + KVL + DR, :])

            for qc in range(4):
                load_chunk(qc, nc.gpsimd)

            # --- q branch: lora kept resident, normalized locally
            sq_q = []
            ql_raw = []

            def q_cb(m, ps):
                qa = qlp.tile([P, TPC], CD, tag=f"qlr{m}", name=f"qlr{m}")
                nc.scalar.copy(qa[:], ps[:])
                ql_raw.append(qa)
                sq = sqp.tile([P, TPC], CD, tag=f"sq{m}", name=f"sq{m}")
                nc.scalar.square(sq[:], ps[:])
                sq_q.append(sq)

            lora_proj(wqaT, QL // P, q_cb)

            # q inv-rms row, broadcast to all 128 partitions via ones matmul
            ss2 = ssps1.tile([1, TPC], F32, tag="ss")
            for m in range(len(sq_q)):
                nc.tensor.matmul(ss2[:], ones_col[:], sq_q[m][:],
                                 start=(m == 0), stop=(m == len(sq_q) - 1))
            srt2 = mp1.tile([1, TPC], F32, tag="srt")
            nc.scalar.activation(srt2[:], ss2[:], AFT.Sqrt,
                                 bias=eps_t[:], scale=1.0 / QL)
            inv_f2 = mp1.tile([1, TPC], F32, tag="invf")
            nc.vector.reciprocal_approx_fast(inv_f2[:], srt2[:])
            invq = mp1.tile([1, TPC], CD, tag="invq")
            nc.vector.tensor_copy(invq[:], inv_f2[:])
            bq_ps = ssps1.tile([P, TPC], F32, tag="bqps")
            nc.tensor.matmul(bq_ps[:], ones_row[:], invq[:], start=True, stop=True)
            bq = qlp.tile([P, TPC], CD, tag="bq")
            nc.scalar.copy(bq[:], bq_ps[:])
            for m in range(QL // P):
                qn = qlp.tile([P, TPC], CD, tag=f"qn{m}", name=f"qn{m}")
                nc.vector.tensor_mul(qn[:], ql_raw[m][:], bq[:])
                qn_t[m] = qn

        # ---------------- q production (all 16 heads, local tokens) --------
        # merged A2A slot layout per dst core c (QSLOT=384 rows):
        #   [nope head 2c (128) | nope head 2c+1 (128) | rope pair c (128)]
        with tc.tile_pool(name="pqw", bufs=3) as wqp, \
             tc.tile_pool(name="pqps", bufs=3, space="PSUM") as psq, \
             tc.tile_pool(name="pqmisc", bufs=2) as mpq, \
             tc.tile_pool(name="pqout", bufs=3) as opq:
            perm_sb = mpq.tile([P, P], CD, tag="perm")
            nc.sync.dma_start(perm_sb[:], perm[:, :])

            def q_prod(m):
                wt = wqp.tile([P, QL], CD, tag="wqb", name=f"wqb{m}")
                nc.sync.dma_start(wt[:], wqbT[m, :, :])
                ps = psq.tile([P, TPC], F32, tag="qps")
                for kb in range(QL // P):
                    nc.tensor.matmul(ps[:], wt[:, kb * P:(kb + 1) * P], qn_t[kb][:],
                                     start=(kb == 0), stop=(kb == QL // P - 1))
                if m < H:
                    qo = opq.tile([P, TPC], CD, tag="qo")
                    nc.scalar.copy(qo[:], ps[:])
                    row = (m // 2) * QSLOT + (m % 2) * P
                    nc.sync.dma_start(a2aq_in[row:row + P, :], qo[:])
                else:
                    j = m - H
                    qr_raw = mpq.tile([P, TPC], CD, tag="qrraw")
                    nc.scalar.copy(qr_raw[:], ps[:])
                    ps_sw = psq.tile([P, TPC], F32, tag="qpsw")
                    nc.tensor.matmul(ps_sw[:], perm_sb[:], qr_raw[:],
                                     start=True, stop=True)
                    t_a = mpq.tile([P, TPC], CD, tag="qra")
                    nc.vector.tensor_mul(t_a[:], qr_raw[:], cosc_q[:])
                    t_b = mpq.tile([P, TPC], CD, tag="qrb")
                    nc.vector.tensor_mul(t_b[:], ps_sw[:], sinc_q[:])
                    qr_fin = opq.tile([P, TPC], CD, tag="qrfin")
                    nc.vector.tensor_add(qr_fin[:], t_a[:], t_b[:])
                    row = j * QSLOT + 2 * P
                    nc.sync.dma_start(a2aq_in[row:row + P, :], qr_fin[:])

            cosc_q = mpq.tile([P, TPC], CD, tag="coscq")
            nc.sync.dma_start(cosc_q[:], cosc[:, :])
            sinc_q = mpq.tile([P, TPC], CD, tag="sincq")
            nc.sync.dma_start(sinc_q[:], sinc[:, :])

            for m in range(NM_Q):
                q_prod(m)
            for qc in range(4, NCH):
                load_chunk(qc, nc.scalar)
            nc.gpsimd.collective_compute(
                "AllToAll", mybir.AluOpType.bypass, replica_groups=groups,
                ins=[a2aq_in.ap().opt()], outs=[a2aq_out.ap().opt()])

        _qlcm.__exit__(None, None, None)
        _wp3cm = tc.tile_pool(name="p3w", bufs=1)
        wp3 = _wp3cm.__enter__()

        # prefetch ALL phase-3 wo weights now (ungated, on sync, BEFORE the
        # A2A-gated q receives so they never queue behind the collective)
        for m in range(DIM // P):
            wt = wp3.tile([P, DIM], CD, tag=f"wo{m}", name=f"wo{m}")
            nc.sync.dma_start(wt[:], woT[m, :, :])
            wo_sb[m] = wt

        # ---------------- phase 2: k/v production (gated on AG_kv) ---------
        if True:
            kvscope = tc.tile_pool(name="p2psk", bufs=1, space="PSUM")
            psk = kvscope.__enter__()
            vscope = tc.tile_pool(name="p2psv", bufs=3, space="PSUM")
            psv = vscope.__enter__()
            bscope = tc.tile_pool(name="p2psb", bufs=1, space="PSUM")
            psb = bscope.__enter__()
            for qc in range(NCH):
                c0 = qc * CHUNK
                actkv_ts = actkv_all[qc * (KVL // P):(qc + 1) * (KVL // P)]
                ikv = ikv_all[qc]
                bkv_ps = psb.tile([P, CHUNK], F32, tag="bkv")
                nc.tensor.matmul(bkv_ps[:], ones_row[:], ikv[:],
                                 start=True, stop=True)
                bkv = nrmp.tile([P, CHUNK], CD, tag="bkvsb")
                nc.scalar.copy(bkv[:], bkv_ps[:])
                kps = [psk.tile([P, CHUNK], F32, tag=f"kps{m}", name=f"kps{m}")
                       for m in range(2)]
                for kb in range(KVL // P):
                    for m in range(2):
                        nc.tensor.matmul(kps[m][:],
                                         wkvk_sb[m][:, kb * P:(kb + 1) * P],
                                         actkv_ts[kb][:],
                                         start=(kb == 0), stop=(kb == KVL // P - 1))
                for h in range(HPC):
                    nc.vector.tensor_mul(k_n[h][:, c0:c0 + CHUNK], kps[h][:], bkv[:])
                for ktc in range(CHUNK // P):
                    # inv_kv as a [128,1] column for this kt tile (K=1 matmul)
                    col_ps = psb.tile([P, 1], F32, tag="colp")
                    nc.tensor.matmul(col_ps[:],
                                     ikv[0:1, ktc * P:(ktc + 1) * P],
                                     one_1x1[:], start=True, stop=True)
                    vcol = nrmp.tile([P, 1], F32, tag="vcol")
                    nc.scalar.copy(vcol[:], col_ps[:])
                    vps = psv.tile([P, HPC * DV], F32, tag="vps")
                    for kb in range(KVL // P):
                        nc.tensor.matmul(vps[:],
                                         actkv_ts[kb][:, ktc * P:(ktc + 1) * P],
                                         wkvv_sb[kb][:],
                                         start=(kb == 0), stop=(kb == KVL // P - 1))
                    nc.scalar.activation(v_tok[qc * (CHUNK // P) + ktc][:], vps[:],
                                         AFT.Copy, bias=0.0, scale=vcol[:])
            bscope.__exit__(None, None, None)
            vscope.__exit__(None, None, None)
            kvscope.__exit__(None, None, None)

            # receive q from the merged A2A into the head-major SBUF tiles
            # (sync queue; wo prefetch already issued ahead of these)
            for r in range(NC):
                base = r * QSLOT
                for h in range(HPC):
                    nc.sync.dma_start(
                        q_n[h][:, r * TPC:(r + 1) * TPC],
                        a2aq_out[base + h * P: base + (h + 1) * P, :])
                    nc.sync.dma_start(
                        q_rope[h][(r // 4) * DR:(r // 4 + 1) * DR,
                                  (r % 4) * TPC:(r % 4 + 1) * TPC],
                        a2aq_out[base + 2 * P + h * DR: base + 2 * P + (h + 1) * DR, :])

            # ----- causal attention (S^T formulation), head-outer for A2A
            # overlap; phase-3 even-head work interleaved after h1
            _rp3cm = tc.tile_pool(name="p3r", bufs=1)
            rp3 = _rp3cm.__enter__()
            rhs_t = [None] * H
            partials = [None] * (DIM // P)
            order = [g for g in range(H) if g % HPC == 0]
            order2 = [g for g in range(H) if g % HPC != 0]

            with tc.tile_pool(name="apt", bufs=4) as ptp, \
                 tc.tile_pool(name="aout", bufs=2, space="PSUM") as outp, \
                 tc.tile_pool(name="aden", bufs=2, space="PSUM") as denp, \
                 tc.tile_pool(name="ast", bufs=4, space="PSUM") as stp, \
                 tc.tile_pool(name="afin", bufs=3) as finp:

                def load_head(g):
                    blk = (g // HPC) * (DV + 1)
                    rt = rp3.tile([P, CHUNK], CD, tag="r", name=f"r{g}", bufs=2)
                    nc.sync.dma_start(rt[:], a2a_out[g % HPC][blk:blk + DV, :])
                    dr = rp3.tile([1, CHUNK], CD, tag="d", name=f"d{g}", bufs=2)
                    nc.sync.dma_start(dr[:],
                                      a2a_out[g % HPC][blk + DV:blk + DV + 1, :])
                    bc_ps = denp.tile([P, CHUNK], F32, tag="den", name=f"bc{g}")
                    nc.tensor.matmul(bc_ps[:], ones_row[:], dr[:],
                                     start=True, stop=True)
                    binv = rp3.tile([P, CHUNK], F32, tag="bi", name=f"bi{g}",
                                    bufs=2)
                    nc.vector.reciprocal_approx_fast(binv[:], bc_ps[:])
                    rn = rp3.tile([P, CHUNK], CD, tag="rn", name=f"rn{g}", bufs=8)
                    nc.vector.tensor_mul(rn[:], rt[:], binv[:])
                    rhs_t[g] = rn

                def partial_group(m, heads, acc):
                    """wo partial for m-tile over the given head groups."""
                    ps = outp.tile([P, CHUNK], F32, tag="out", name=f"p3ps{m}_{acc}")
                    for i, g in enumerate(heads):
                        nc.tensor.matmul(ps[:], wo_sb[m][:, g * P:(g + 1) * P],
                                         rhs_t[g][:],
                                         start=(i == 0), stop=(i == len(heads) - 1))
                    return ps

                def attn_block(h, b, qcl):
                    qg = b * (S // CHUNK) + qcl
                    q0 = qg * CHUNK
                    nkt = (CHUNK // P) * (qcl + 1)
                    out_ps = outp.tile([P, CHUNK], F32, tag="out", name=f"out{h}_{qg}")
                    den_ps = denp.tile([1, CHUNK], F32, tag="den", name=f"den{h}_{qg}")

                    def qk_stage(kt):
                        kcol = b * S + kt * P
                        st_ps = stp.tile([P, CHUNK], F32, tag="st",
                                         name=f"st{h}_{qg}_{kt}")
                        nc.tensor.matmul(st_ps[:],
                                         k_n[h][:, kcol:kcol + P],
                                         q_n[h][:, q0:q0 + CHUNK],
                                         start=True, stop=False)
                        nc.tensor.matmul(st_ps[:],
                                         k_pe[b * DR:(b + 1) * DR, kt * P:(kt + 1) * P],
                                         q_rope[h][b * DR:(b + 1) * DR,
                                                   qcl * CHUNK:(qcl + 1) * CHUNK],
                                         start=False, stop=True)
                        pt = ptp.tile([P, CHUNK], CD, tag="pt",
                                      name=f"pt{h}_{qg}_{kt}", bufs=5)
                        nc.scalar.activation(pt[:], st_ps[:], AFT.Exp,
                                             bias=0.0, scale=SCALE)
                        if kt >= (CHUNK // P) * qcl:  # diagonal block
                            ptm = ptp.tile([P, CHUNK], CD, tag="ptm",
                                           name=f"ptm{h}_{qg}_{kt}", bufs=5)
                            nc.gpsimd.affine_select(
                                ptm[:], pt[:], pattern=[[1, CHUNK]],
                                base=qcl * CHUNK - kt * P,
                                channel_multiplier=-1,
                                compare_op=mybir.AluOpType.is_ge, fill=0.0)
                            return ptm
                        return pt

                    pend = {}

                    def av_stage(kt, ptf):
                        nc.tensor.matmul(
                            out_ps[:],
                            v_tok[(b * S // P) + kt][:, h * DV:(h + 1) * DV],
                            ptf[:],
                            start=(kt == 0), stop=(kt == nkt - 1))
                        # 4-way tree-summed denominator
                        r = kt % 4
                        if r in (0, 2):
                            pend[r] = ptf
                        elif r == 1:
                            pd = ptp.tile([P, CHUNK], CD, tag="pden",
                                          name=f"pden{h}_{qg}_{kt}", bufs=2)
                            nc.vector.tensor_add(pd[:], pend[0][:], ptf[:])
                            pend[1] = pd
                        else:
                            pd2 = ptp.tile([P, CHUNK], CD, tag="pden",
                                           name=f"pden{h}_{qg}_{kt}", bufs=2)
                            nc.vector.tensor_add(pd2[:], pend[2][:], ptf[:])
                            pd4 = ptp.tile([P, CHUNK], CD, tag="pden4",
                                           name=f"pden4_{h}_{qg}_{kt}", bufs=2)
                            nc.vector.tensor_add(pd4[:], pend[1][:], pd2[:])
                            nc.tensor.matmul(den_ps[:], ones_col[:], pd4[:],
                                             start=(kt == 3),
                                             stop=(kt == nkt - 1))

                    pipe = []
                    for kt in range(nkt):
                        pipe.append(qk_stage(kt))
                        if len(pipe) > 2:
                            av_stage(kt - 2, pipe.pop(0))
                    for j, ptf in enumerate(pipe):
                        av_stage(nkt - len(pipe) + j, ptf)
                    attn = finp.tile([P, CHUNK], CD, tag="attn", bufs=3)
                    nc.vector.tensor_copy(attn[:], out_ps[:])
                    den_c = finp.tile([1, CHUNK], CD, tag="denc", bufs=3)
                    nc.scalar.copy(den_c[:], den_ps[:])
                    nc.sync.dma_start(
                        a2a_in[h][qg * (DV + 1):qg * (DV + 1) + DV, :], attn[:])
                    nc.sync.dma_start(
                        a2a_in[h][qg * (DV + 1) + DV:(qg + 1) * (DV + 1), :],
                        den_c[:])

                # ---- h0: attention, then its A2A
                for b in range(B):
                    for qcl in range(S // CHUNK):
                        attn_block(0, b, qcl)
                nc.gpsimd.collective_compute(
                    "AllToAll", mybir.AluOpType.bypass, replica_groups=groups,
                    ins=[a2a_in[0].ap().opt()], outs=[a2a_out[0].ap().opt()])

                # ---- h1: attention, then its A2A
                for b in range(B):
                    for qcl in range(S // CHUNK):
                        attn_block(1, b, qcl)
                nc.gpsimd.collective_compute(
                    "AllToAll", mybir.AluOpType.bypass, replica_groups=groups,
                    ins=[a2a_in[1].ap().opt()], outs=[a2a_out[1].ap().opt()])

                # ---- phase-3 even half: runs during the h1 A2A flight
                for g in order:
                    load_head(g)
                for m in range(DIM // P):
                    ps = partial_group(m, order, 0)
                    pa = rp3.tile([P, CHUNK], CD, tag=f"pa{m}", name=f"pa{m}")
                    nc.scalar.copy(pa[:], ps[:])
                    partials[m] = pa

                # ---- phase-3 odd half (tail)
                for g in order2:
                    load_head(g)
                with tc.tile_pool(name="p3o", bufs=2) as op3:
                    for m in range(DIM // P):
                        ps = partial_group(m, order2, 1)
                        ot = op3.tile([P, CHUNK], F32, tag="ot")
                        nc.vector.tensor_add(ot[:], ps[:], partials[m][:])
                        nc.sync.dma_start(outT[m * P:(m + 1) * P, :], ot[:])
            _rp3cm.__exit__(None, None, None)

        _wp3cm.__exit__(None, None, None)
        _nrmcm.__exit__(None, None, None)
        _akvcm.__exit__(None, None, None)
        _wp2cm.__exit__(None, None, None)
        _vp2cm.__exit__(None, None, None)
        _kp2cm.__exit__(None, None, None)
        _qp2cm.__exit__(None, None, None)

    nc.compile()
    return nc


def _tile_kxm(w, nk, nm):
    """(nk*128, nm*128) -> (nm, 128, nk*128): [m][p][kt*128+j] = w[kt*128+p, m*128+j]."""
    return np.ascontiguousarray(
        w.reshape(nk, P, nm, P).transpose(2, 1, 0, 3).reshape(nm, P, nk * P))


_CACHE = {}


def _prep(inputs):
    x = np.asarray(inputs["x"], np.float32)
    fc = np.asarray(inputs["freqs_cos"], np.float32)
    fs = np.asarray(inputs["freqs_sin"], np.float32)
    wq_a = np.asarray(inputs["wq_a"], np.float32)
    q_norm_w = np.asarray(inputs["q_norm_w"], np.float32)
    wq_b = np.asarray(inputs["wq_b"], np.float32)
    wkv_a = np.asarray(inputs["wkv_a"], np.float32)
    kv_norm_w = np.asarray(inputs["kv_norm_w"], np.float32)
    wkv_b = np.asarray(inputs["wkv_b"], np.float32)
    wo = np.asarray(inputs["wo"], np.float32)

    x_flat = x.reshape(T, DIM)

    wqaT_t = _tile_kxm(wq_a.T, DIM // P, QL // P)

    at = wkv_a.T                                     # (DIM, 576)
    Rw = at[:, KVL::2]
    Iw = at[:, KVL + 1::2]
    wkvaT_t = _tile_kxm(np.concatenate([at[:, :KVL], Rw, Iw, Iw, Rw], axis=1),
                        DIM // P, KVE // P)

    wqb_sT = (wq_b * q_norm_w[None, :]).T            # (QL, H*192)
    wkvb_sT = (wkv_b * kv_norm_w[None, :]).T         # (KVL, H*256)

    woT_t = _tile_kxm(wo.T, DIM // P, DIM // P)

    cT, sT = fc.T, fs.T
    cosbM = np.concatenate([cT, cT, cT, cT], axis=0)   # (128, S)
    sinbM = np.concatenate([-sT, sT, -sT, sT], axis=0)
    permM = np.zeros((P, P), np.float32)
    permM[np.arange(P) ^ 32, np.arange(P)] = 1.0

    # wqb for ALL heads: 16 nope m-tiles then 8 rope m-tiles (head pairs,
    # de-interleaved [R_2j | I_2j | R_2j+1 | I_2j+1])
    qb = [wqb_sT[:, h * 192: h * 192 + DN] for h in range(H)]
    for j in range(H // 2):
        for h in (2 * j, 2 * j + 1):
            rope = wqb_sT[:, h * 192 + DN:(h + 1) * 192]
            qb.append(rope[:, 0::2])
            qb.append(rope[:, 1::2])
    wqbT_t = _tile_kxm(np.concatenate(qb, axis=1), QL // P, NM_Q)

    in_maps = []
    for c in range(NC):
        h0, h1 = 2 * c, 2 * c + 1
        kn = [wkvb_sT[:, h * 256: h * 256 + DN] for h in (h0, h1)]
        vv = [wkvb_sT[:, h * 256 + DN: (h + 1) * 256] for h in (h0, h1)]
        wkvbTk_c = _tile_kxm(np.concatenate(kn, axis=1), KVL // P, 2)
        wkvbTv_c = np.ascontiguousarray(np.concatenate(vv, axis=1))

        pos0 = (c * TPC) % S
        in_maps.append({
            "xT": np.ascontiguousarray(x_flat[c * TPC:(c + 1) * TPC].T).astype(BF),
            "wqaT": wqaT_t.astype(BF), "wkvaT": wkvaT_t.astype(BF),
            "wqbT": wqbT_t.astype(BF), "wkvbTk": wkvbTk_c.astype(BF),
            "wkvbTv": wkvbTv_c.astype(BF),
            "woT": woT_t.astype(BF), "perm": permM.astype(BF),
            "cosc": np.ascontiguousarray(cosbM[:, pos0:pos0 + TPC]).astype(BF),
            "sinc": np.ascontiguousarray(sinbM[:, pos0:pos0 + TPC]).astype(BF),
        })
    return in_maps


def kernel(**inputs):
    in_maps = _prep(inputs)
    if "nc" not in _CACHE:
        _CACHE["nc"] = build_nc()
    r = run_bass_kernel_spmd(_CACHE["nc"], in_maps, list(range(NC)))
    out_flat = np.empty((T, DIM), np.float32)
    for c in range(NC):
        out_flat[c * TPC:(c + 1) * TPC] = r.results[c]["outT"].T
    return out_flat.reshape(B, S, DIM)


# revision 16
# speedup vs baseline: 1.0937x; 1.0937x over previous
"""MLA (DeepSeek-style multi-head latent attention) distributed Bass kernel
for 8 TRN2 NeuronCores — v3 (schedule restructure).

Problem shapes (hardcoded):
  x (2, 2048, 2048), DIM=2048, N_HEADS=16, Q_LORA=1536, KV_LORA=512,
  QK_NOPE=128, QK_ROPE=64, V_HEAD=128, causal SDPA, scale=192**-0.5.

v3 changes vs v2 (which ran 564us with ~50us of Tensor stalls):
  * warmup collective dispatched first so the one-time CC barrier/rendezvous
    overlaps phase-1 compute instead of delaying the kv AllGather.
  * ONE merged q AllToAll (nope for all 16 heads + rope pairs, 1.5MB/core
    eff) replacing two serialized A2As with an 18us CC-stream hole.
  * queue hygiene: scalar queue carries ACT compute only; all AG-gated
    gathered-kv loads live on the gpsimd queue (naturally ordered after the
    AG dispatch), so no head-of-line blocking of phase-1 copies; wo
    prefetch issues on sync BEFORE the (A2A-gated) q receives.
  * phase-2 k/v production sits between q production and attention in the
    Tensor stream, covering the q A2A flight; attention starts as soon as
    the receives land.

phase 1 (token-parallel, 512 tokens/core):
  kv branch first: kv latent + roped k_pe + inv-rms row -> AllGather
  (577 rows) hidden under the q branch.
  q branch: q_lora -> rmsnorm locally -> full wq_b projection for ALL
  16 heads + rope -> merged AllToAll shipping each peer's 2 heads
  (nope) + its rope pair.
phase 2 (head-parallel, 2 heads/core): k/v production from the gathered
  latent (covers the q A2A flight), then causal flash attention (S^T
  formulation, exp without max-subtraction).
phase 3: wo partials; even half interleaved after h1 attention during the
  second out-A2A flight; odd half as the tail. AllToAll per local head
  ships UNNORMALIZED attention outputs + softmax denominators;
  normalization happens receiver-side.

All matmul operands are bfloat16 (fp32 PSUM accumulation); fp32 for the
rms/softmax statistics chains. Activations are feature-major
[features(partitions), tokens(free)] -- no transposes anywhere.
"""
import sys

sys.path.insert(0, "/opt/trn_rl_repo")

import numpy as np
import ml_dtypes

import concourse.bacc as bacc
import concourse.mybir as mybir
import concourse.tile as tile
from concourse.bass_utils import run_bass_kernel_spmd

BF = ml_dtypes.bfloat16
F32 = mybir.dt.float32
CD = mybir.dt.bfloat16
AFT = mybir.ActivationFunctionType

DIM = 2048
H = 16
QL = 1536
KVL = 512
DN = 128          # qk_nope
DR = 64           # qk_rope
DV = 128          # v head dim
B, S = 2, 2048
T = B * S
NC = 8
TPC = T // NC     # 512 tokens per core
HPC = H // NC     # 2 heads per core
SCALE = (DN + DR) ** -0.5
EPS = 1e-6
P = 128
CHUNK = 512
NCH = T // CHUNK
KVE = KVL + 2 * DR   # 640 phase-1 kv output cols (incl swapped-rope block)
KVG = KVL + DR + 1   # 577 gathered kv rows (+ inv_rms row)
NM_Q = H + H // 2    # 24 q-production m tiles (16 nope + 8 rope)


def build_nc():
    nc = bacc.Bacc("TRN2", target_bir_lowering=False, debug=False, num_devices=NC)

    xT = nc.declare_dram_parameter("xT", [DIM, TPC], CD, isOutput=False)
    wqaT = nc.declare_dram_parameter("wqaT", [QL // P, P, DIM], CD, isOutput=False)
    wkvaT = nc.declare_dram_parameter("wkvaT", [KVE // P, P, DIM], CD, isOutput=False)
    wqbT = nc.declare_dram_parameter("wqbT", [NM_Q, P, QL], CD, isOutput=False)
    wkvbTk = nc.declare_dram_parameter("wkvbTk", [2, P, KVL], CD, isOutput=False)
    wkvbTv = nc.declare_dram_parameter("wkvbTv", [KVL, 2 * DV], CD, isOutput=False)
    woT = nc.declare_dram_parameter("woT", [DIM // P, P, DIM], CD, isOutput=False)
    cosc = nc.declare_dram_parameter("cosc", [P, TPC], CD, isOutput=False)
    sinc = nc.declare_dram_parameter("sinc", [P, TPC], CD, isOutput=False)
    perm = nc.declare_dram_parameter("perm", [P, P], CD, isOutput=False)
    outT = nc.declare_dram_parameter("outT", [DIM, TPC], F32, isOutput=True)

    agkv_in = nc.dram_tensor("agkv_in", [KVG, TPC], CD)
    agkv_out = nc.dram_tensor("agkv_out", [NC * KVG, TPC], CD, addr_space="Shared")
    # q A2A split in two so attention h0 (even head) can start while the
    # odd-head payload is still in flight: A2A-a = even-head nope + rope
    # pairs (256 rows/slot), A2A-b = odd-head nope (128 rows/slot)
    a2aqa_in = nc.dram_tensor("a2aqa_in", [NC * 2 * P, TPC], CD)
    a2aqa_out = nc.dram_tensor("a2aqa_out", [NC * 2 * P, TPC], CD)
    a2aqb_in = nc.dram_tensor("a2aqb_in", [NC * P, TPC], CD)
    a2aqb_out = nc.dram_tensor("a2aqb_out", [NC * P, TPC], CD)
    a2a_in = [nc.dram_tensor(f"a2a_in{h}", [NC * (DV + 1), CHUNK], CD)
              for h in range(HPC)]
    a2a_out = [nc.dram_tensor(f"a2a_out{h}", [NC * (DV + 1), CHUNK], CD)
               for h in range(HPC)]
    groups = [list(range(NC))]

    with tile.TileContext(nc) as tc, \
         tc.tile_pool(name="const", bufs=1) as constp:
        ones_f = constp.tile([P, 1], F32, tag="onesf")
        nc.any.memset(ones_f[:], 1.0)
        ones_col = constp.tile([P, 1], CD, tag="onesc")
        nc.vector.tensor_copy(ones_col[:], ones_f[:])
        onesr_f = constp.tile([1, P], F32, tag="onesrf")
        nc.any.memset(onesr_f[:], 1.0)
        ones_row = constp.tile([1, P], CD, tag="onesr")
        nc.vector.tensor_copy(ones_row[:], onesr_f[:])
        one_f11 = constp.tile([1, 1], F32, tag="onef11")
        nc.any.memset(one_f11[:], 1.0)
        one_1x1 = constp.tile([1, 1], CD, tag="one11")
        nc.vector.tensor_copy(one_1x1[:], one_f11[:])
        eps_t = constp.tile([1, 1], F32, tag="eps")
        nc.any.memset(eps_t[:], EPS)
        # touch Exp once now so the ACT table load happens during phase 1,
        # not at the first attention softmax
        warm_e = constp.tile([1, 1], F32, tag="warme")
        nc.scalar.activation(warm_e[:], eps_t[:], AFT.Exp, bias=0.0, scale=1.0)

        wo_sb = [None] * (DIM // P)

        # persistent phase-2 tensors
        _qp2cm = tc.tile_pool(name="p2q", bufs=1)
        qp2 = _qp2cm.__enter__()
        _kp2cm = tc.tile_pool(name="p2k", bufs=1)
        kp2 = _kp2cm.__enter__()
        _vp2cm = tc.tile_pool(name="p2v", bufs=T // P)
        vp2 = _vp2cm.__enter__()
        q_n = [qp2.tile([P, T], CD, tag=f"qn{h}", name=f"qn{h}") for h in range(HPC)]
        q_rope = [qp2.tile([2 * DR, S], CD, tag=f"qrope{h}", name=f"qrope{h}")
                  for h in range(HPC)]
        k_n = [kp2.tile([P, T], CD, tag=f"kn{h}", name=f"kn{h}") for h in range(HPC)]
        k_pe = kp2.tile([2 * DR, S], CD, tag="kpe")
        v_tok = [vp2.tile([P, HPC * DV], CD, tag="vtok", name=f"vtok{i}")
                 for i in range(T // P)]

        # phase-2 load pools (created early for stack-order discipline)
        _wp2cm = tc.tile_pool(name="p2w", bufs=1)
        wp2 = _wp2cm.__enter__()
        _akvcm = tc.tile_pool(name="p2actkv", bufs=1)
        actkvp = _akvcm.__enter__()
        _nrmcm = tc.tile_pool(name="p2nrm", bufs=2)
        nrmp = _nrmcm.__enter__()
        wkvk_sb = []
        wkvv_sb = []
        actkv_all = []
        ikv_all = []

        # ---------------- phase 1 ------------------------------------------
        _qlcm = tc.tile_pool(name="p1ql", bufs=1)
        qlp = _qlcm.__enter__()
        qn_t = [None] * (QL // P)

        with tc.tile_pool(name="p1x", bufs=1) as xp, \
             tc.tile_pool(name="p1w", bufs=3) as wp1, \
             tc.tile_pool(name="p1ps", bufs=3, space="PSUM") as ps1, \
             tc.tile_pool(name="p1ssps", bufs=2, space="PSUM") as ssps1, \
             tc.tile_pool(name="p1sq", bufs=1) as sqp, \
             tc.tile_pool(name="p1misc", bufs=2) as mp1:

            # first kv weight tile before x so the first matmul's operands
            # land as early as possible
            wkv0 = wp1.tile([P, DIM], CD, tag="w1", name="wkv0")
            nc.sync.dma_start(wkv0[:], wkvaT[0, :, :])
            xts = []
            for kb in range(DIM // P):
                xt = xp.tile([P, TPC], CD, tag=f"x{kb}")
                nc.sync.dma_start(xt[:], xT[kb * P:(kb + 1) * P, :])
                xts.append(xt)

            cosc_sb = mp1.tile([P, TPC], CD, tag="cosc")
            nc.sync.dma_start(cosc_sb[:], cosc[:, :])
            sinc_sb = mp1.tile([P, TPC], CD, tag="sinc")
            nc.sync.dma_start(sinc_sb[:], sinc[:, :])

            def lora_proj(w_param, nm, out_cb, w0=None):
                for m in range(nm):
                    if m == 0 and w0 is not None:
                        wt = w0
                    else:
                        wt = wp1.tile([P, DIM], CD, tag="w1", name=f"w1_{m}")
                        nc.sync.dma_start(wt[:], w_param[m, :, :])
                    ps = ps1.tile([P, TPC], F32, tag="ps1")
                    for kb in range(DIM // P):
                        nc.tensor.matmul(ps[:], wt[:, kb * P:(kb + 1) * P], xts[kb][:],
                                         start=(kb == 0), stop=(kb == DIM // P - 1))
                    out_cb(m, ps)

            # --- kv branch (payload shipped unnormalized + inv-rms row)
            sq_kv = []

            def kv_cb(m, ps):
                if m < 4:
                    kv = mp1.tile([P, TPC], CD, tag="pay")
                    nc.scalar.copy(kv[:], ps[:])
                    nc.sync.dma_start(agkv_in[m * P:(m + 1) * P, :], kv[:])
                    sq = sqp.tile([P, TPC], CD, tag=f"skv{m}", name=f"skv{m}")
                    nc.scalar.square(sq[:], ps[:])
                    sq_kv.append(sq)
                else:
                    # rows 0:64 deinterleaved kpe [r;i], rows 64:128 [i;r]
                    t_a = mp1.tile([DR, TPC], CD, tag="ropea")
                    nc.vector.tensor_mul(t_a[:], ps[0:DR, :], cosc_sb[0:DR, :])
                    t_b = mp1.tile([DR, TPC], CD, tag="ropeb")
                    nc.vector.tensor_mul(t_b[:], ps[DR:2 * DR, :], sinc_sb[0:DR, :])
                    kpe = mp1.tile([DR, TPC], CD, tag="ropeo")
                    nc.vector.tensor_add(kpe[:], t_a[:], t_b[:])
                    nc.sync.dma_start(agkv_in[KVL:KVL + DR, :], kpe[:])

            lora_proj(wkvaT, KVE // P, kv_cb, w0=wkv0)
            # kv inv-rms row
            ss = ssps1.tile([1, TPC], F32, tag="ss")
            for m in range(len(sq_kv)):
                nc.tensor.matmul(ss[:], ones_col[:], sq_kv[m][:],
                                 start=(m == 0), stop=(m == len(sq_kv) - 1))
            srt = mp1.tile([1, TPC], F32, tag="srt")
            nc.scalar.activation(srt[:], ss[:], AFT.Sqrt,
                                 bias=eps_t[:], scale=1.0 / KVL)
            inv_f = mp1.tile([1, TPC], F32, tag="invf")
            nc.vector.reciprocal_approx_fast(inv_f[:], srt[:])
            inv = mp1.tile([1, TPC], CD, tag="inv")
            nc.vector.tensor_copy(inv[:], inv_f[:])
            nc.sync.dma_start(agkv_in[KVL + DR:KVL + DR + 1, :], inv[:])

            nc.gpsimd.collective_compute(
                "AllGather", mybir.AluOpType.bypass, replica_groups=groups,
                ins=[agkv_in.ap().opt()], outs=[agkv_out.ap().opt()])

            # hoisted phase-2 input loads on the GPSIMD queue: they are
            # gated on the AllGather anyway and gpsimd has nothing else
            # queued, so they cause no head-of-line blocking
            for m in range(2):
                w = wp2.tile([P, KVL], CD, tag=f"wkvk{m}", name=f"wkvk{m}")
                nc.gpsimd.dma_start(w[:], wkvbTk[m, :, :])
                wkvk_sb.append(w)
            for kb in range(KVL // P):
                w = wp2.tile([P, 2 * DV], CD, tag=f"wkvv{kb}", name=f"wkvv{kb}")
                nc.gpsimd.dma_start(w[:], wkvbTv[kb * P:(kb + 1) * P, :])
                wkvv_sb.append(w)

            def load_chunk(qc):
                # all gathered-kv loads live on the gpsimd queue: they are
                # gated on the AllGather anyway and nothing compute-critical
                # queues there, so no head-of-line blocking. Chunks 0-4 have
                # fresh buffers (bufs=20); chunks 5-7 reuse chunk 0-2
                # buffers and are issued AFTER the q A2A dispatches so their
                # WAR waits cannot delay the collectives.
                for kb in range(KVL // P):
                    at = actkvp.tile([P, CHUNK], CD, tag="actkv",
                                     name=f"actkv{qc}_{kb}", bufs=20)
                    nc.gpsimd.dma_start(
                        at[:],
                        agkv_out[qc * KVG + kb * P: qc * KVG + (kb + 1) * P, :])
                    actkv_all.append(at)
                ik = actkvp.tile([1, CHUNK], CD, tag="ikv", name=f"ikv{qc}",
                                 bufs=4)
                nc.gpsimd.dma_start(
                    ik[:],
                    agkv_out[qc * KVG + KVL + DR: qc * KVG + KVL + DR + 1, :])
                ikv_all.append(ik)
                nc.gpsimd.dma_start(
                    k_pe[(qc // 4) * DR:(qc // 4 + 1) * DR,
                         (qc % 4) * CHUNK:(qc % 4 + 1) * CHUNK],
                    agkv_out[qc * KVG + KVL: qc * KVG + KVL + DR, :])

            for qc in range(5):
                load_chunk(qc)

            # --- q branch: lora kept resident, normalized locally
            sq_q = []
            ql_raw = []

            def q_cb(m, ps):
                qa = qlp.tile([P, TPC], CD, tag=f"qlr{m}", name=f"qlr{m}")
                nc.scalar.copy(qa[:], ps[:])
                ql_raw.append(qa)
                sq = sqp.tile([P, TPC], CD, tag=f"sq{m}", name=f"sq{m}")
                nc.scalar.square(sq[:], ps[:])
                sq_q.append(sq)

            lora_proj(wqaT, QL // P, q_cb)

            # q inv-rms row, broadcast to all 128 partitions via ones matmul
            ss2 = ssps1.tile([1, TPC], F32, tag="ss")
            for m in range(len(sq_q)):
                nc.tensor.matmul(ss2[:], ones_col[:], sq_q[m][:],
                                 start=(m == 0), stop=(m == len(sq_q) - 1))
            srt2 = mp1.tile([1, TPC], F32, tag="srt")
            nc.scalar.activation(srt2[:], ss2[:], AFT.Sqrt,
                                 bias=eps_t[:], scale=1.0 / QL)
            inv_f2 = mp1.tile([1, TPC], F32, tag="invf")
            nc.vector.reciprocal_approx_fast(inv_f2[:], srt2[:])
            invq = mp1.tile([1, TPC], CD, tag="invq")
            nc.vector.tensor_copy(invq[:], inv_f2[:])
            bq_ps = ssps1.tile([P, TPC], F32, tag="bqps")
            nc.tensor.matmul(bq_ps[:], ones_row[:], invq[:], start=True, stop=True)
            bq = qlp.tile([P, TPC], CD, tag="bq")
            nc.scalar.copy(bq[:], bq_ps[:])
            for m in range(QL // P):
                qn = qlp.tile([P, TPC], CD, tag=f"qn{m}", name=f"qn{m}")
                nc.vector.tensor_mul(qn[:], ql_raw[m][:], bq[:])
                qn_t[m] = qn

        # ---------------- q production (all 16 heads, local tokens) --------
        # A2A-a slot per dst core c (256 rows): [nope head 2c | rope pair c]
        # A2A-b slot per dst core c (128 rows): [nope head 2c+1]
        with tc.tile_pool(name="pqw", bufs=2) as wqp, \
             tc.tile_pool(name="pqps", bufs=3, space="PSUM") as psq, \
             tc.tile_pool(name="pqmisc", bufs=1) as mpq, \
             tc.tile_pool(name="pqout", bufs=3) as opq:
            perm_sb = mpq.tile([P, P], CD, tag="perm")
            nc.sync.dma_start(perm_sb[:], perm[:, :])

            def q_prod(m):
                wt = wqp.tile([P, QL], CD, tag="wqb", name=f"wqb{m}")
                nc.sync.dma_start(wt[:], wqbT[m, :, :])
                ps = psq.tile([P, TPC], F32, tag="qps")
                for kb in range(QL // P):
                    nc.tensor.matmul(ps[:], wt[:, kb * P:(kb + 1) * P], qn_t[kb][:],
                                     start=(kb == 0), stop=(kb == QL // P - 1))
                if m < H:
                    qo = opq.tile([P, TPC], CD, tag="qo")
                    nc.scalar.copy(qo[:], ps[:])
                    if m % 2 == 0:
                        row = (m // 2) * 2 * P
                        nc.sync.dma_start(a2aqa_in[row:row + P, :], qo[:])
                    else:
                        row = (m // 2) * P
                        nc.sync.dma_start(a2aqb_in[row:row + P, :], qo[:])
                else:
                    j = m - H
                    qr_raw = mpq.tile([P, TPC], CD, tag="qrraw")
                    nc.scalar.copy(qr_raw[:], ps[:])
                    ps_sw = psq.tile([P, TPC], F32, tag="qpsw")
                    nc.tensor.matmul(ps_sw[:], perm_sb[:], qr_raw[:],
                                     start=True, stop=True)
                    t_a = mpq.tile([P, TPC], CD, tag="qra")
                    nc.vector.tensor_mul(t_a[:], qr_raw[:], cosc_q[:])
                    t_b = mpq.tile([P, TPC], CD, tag="qrb")
                    nc.vector.tensor_mul(t_b[:], ps_sw[:], sinc_q[:])
                    qr_fin = opq.tile([P, TPC], CD, tag="qrfin")
                    nc.vector.tensor_add(qr_fin[:], t_a[:], t_b[:])
                    row = j * 2 * P + P
                    nc.sync.dma_start(a2aqa_in[row:row + P, :], qr_fin[:])

            cosc_q = mpq.tile([P, TPC], CD, tag="coscq")
            nc.sync.dma_start(cosc_q[:], cosc[:, :])
            sinc_q = mpq.tile([P, TPC], CD, tag="sincq")
            nc.sync.dma_start(sinc_q[:], sinc[:, :])

            # even-head nope + rope first, so A2A-a ships while the odd
            # heads are still being produced
            for m in list(range(0, H, 2)) + list(range(H, NM_Q)):
                q_prod(m)
            nc.gpsimd.collective_compute(
                "AllToAll", mybir.AluOpType.bypass, replica_groups=groups,
                ins=[a2aqa_in.ap().opt()], outs=[a2aqa_out.ap().opt()])
            for m in range(1, H, 2):
                q_prod(m)
            nc.gpsimd.collective_compute(
                "AllToAll", mybir.AluOpType.bypass, replica_groups=groups,
                ins=[a2aqb_in.ap().opt()], outs=[a2aqb_out.ap().opt()])
            for qc in range(5, NCH):
                load_chunk(qc)

        _qlcm.__exit__(None, None, None)
        _wp3cm = tc.tile_pool(name="p3w", bufs=1)
        wp3 = _wp3cm.__enter__()

        # prefetch ALL phase-3 wo weights now (ungated, on sync, BEFORE the
        # A2A-gated q receives so they never queue behind the collective)
        for m in range(DIM // P):
            wt = wp3.tile([P, DIM], CD, tag=f"wo{m}", name=f"wo{m}")
            nc.sync.dma_start(wt[:], woT[m, :, :])
            wo_sb[m] = wt

        # ---------------- phase 2: k/v production (gated on AG_kv) ---------
        if True:
            kvscope = tc.tile_pool(name="p2psk", bufs=1, space="PSUM")
            psk = kvscope.__enter__()
            vscope = tc.tile_pool(name="p2psv", bufs=3, space="PSUM")
            psv = vscope.__enter__()
            bscope = tc.tile_pool(name="p2psb", bufs=1, space="PSUM")
            psb = bscope.__enter__()
            for qc in range(NCH):
                c0 = qc * CHUNK
                actkv_ts = actkv_all[qc * (KVL // P):(qc + 1) * (KVL // P)]
                ikv = ikv_all[qc]
                bkv_ps = psb.tile([P, CHUNK], F32, tag="bkv")
                nc.tensor.matmul(bkv_ps[:], ones_row[:], ikv[:],
                                 start=True, stop=True)
                bkv = nrmp.tile([P, CHUNK], CD, tag="bkvsb")
                nc.scalar.copy(bkv[:], bkv_ps[:])
                kps = [psk.tile([P, CHUNK], F32, tag=f"kps{m}", name=f"kps{m}")
                       for m in range(2)]
                for kb in range(KVL // P):
                    for m in range(2):
                        nc.tensor.matmul(kps[m][:],
                                         wkvk_sb[m][:, kb * P:(kb + 1) * P],
                                         actkv_ts[kb][:],
                                         start=(kb == 0), stop=(kb == KVL // P - 1))
                for h in range(HPC):
                    nc.vector.tensor_mul(k_n[h][:, c0:c0 + CHUNK], kps[h][:], bkv[:])
                for ktc in range(CHUNK // P):
                    # inv_kv as a [128,1] column for this kt tile (K=1 matmul)
                    col_ps = psb.tile([P, 1], F32, tag="colp")
                    nc.tensor.matmul(col_ps[:],
                                     ikv[0:1, ktc * P:(ktc + 1) * P],
                                     one_1x1[:], start=True, stop=True)
                    vcol = nrmp.tile([P, 1], F32, tag="vcol")
                    nc.scalar.copy(vcol[:], col_ps[:])
                    vps = psv.tile([P, HPC * DV], F32, tag="vps")
                    for kb in range(KVL // P):
                        nc.tensor.matmul(vps[:],
                                         actkv_ts[kb][:, ktc * P:(ktc + 1) * P],
                                         wkvv_sb[kb][:],
                                         start=(kb == 0), stop=(kb == KVL // P - 1))
                    nc.scalar.activation(v_tok[qc * (CHUNK // P) + ktc][:], vps[:],
                                         AFT.Copy, bias=0.0, scale=vcol[:])
            bscope.__exit__(None, None, None)
            vscope.__exit__(None, None, None)
            kvscope.__exit__(None, None, None)

            # receive q from the split A2As into the head-major SBUF tiles
            # (sync queue; wo prefetch already issued ahead of these)
            for r in range(NC):
                nc.sync.dma_start(
                    q_n[0][:, r * TPC:(r + 1) * TPC],
                    a2aqa_out[r * 2 * P: r * 2 * P + P, :])
                nc.sync.dma_start(
                    q_n[1][:, r * TPC:(r + 1) * TPC],
                    a2aqb_out[r * P: (r + 1) * P, :])
                for h in range(HPC):
                    nc.sync.dma_start(
                        q_rope[h][(r // 4) * DR:(r // 4 + 1) * DR,
                                  (r % 4) * TPC:(r % 4 + 1) * TPC],
                        a2aqa_out[r * 2 * P + P + h * DR:
                                  r * 2 * P + P + (h + 1) * DR, :])

            # ----- causal attention (S^T formulation), head-outer for A2A
            # overlap; phase-3 even-head work interleaved after h1
            _rp3cm = tc.tile_pool(name="p3r", bufs=1)
            rp3 = _rp3cm.__enter__()
            rhs_t = [None] * H
            partials = [None] * (DIM // P)
            order = [g for g in range(H) if g % HPC == 0]
            order2 = [g for g in range(H) if g % HPC != 0]

            with tc.tile_pool(name="apt", bufs=4) as ptp, \
                 tc.tile_pool(name="aout", bufs=2, space="PSUM") as outp, \
                 tc.tile_pool(name="aden", bufs=2, space="PSUM") as denp, \
                 tc.tile_pool(name="ast", bufs=4, space="PSUM") as stp, \
                 tc.tile_pool(name="afin", bufs=3) as finp:

                def load_head(g):
                    blk = (g // HPC) * (DV + 1)
                    rt = rp3.tile([P, CHUNK], CD, tag="r", name=f"r{g}", bufs=2)
                    nc.sync.dma_start(rt[:], a2a_out[g % HPC][blk:blk + DV, :])
                    dr = rp3.tile([1, CHUNK], CD, tag="d", name=f"d{g}", bufs=2)
                    nc.sync.dma_start(dr[:],
                                      a2a_out[g % HPC][blk + DV:blk + DV + 1, :])
                    bc_ps = denp.tile([P, CHUNK], F32, tag="den", name=f"bc{g}")
                    nc.tensor.matmul(bc_ps[:], ones_row[:], dr[:],
                                     start=True, stop=True)
                    binv = rp3.tile([P, CHUNK], F32, tag="bi", name=f"bi{g}",
                                    bufs=1)
                    nc.vector.reciprocal_approx_fast(binv[:], bc_ps[:])
                    rn = rp3.tile([P, CHUNK], CD, tag="rn", name=f"rn{g}", bufs=8)
                    nc.vector.tensor_mul(rn[:], rt[:], binv[:])
                    rhs_t[g] = rn

                def partial_group(m, heads, acc):
                    """wo partial for m-tile over the given head groups."""
                    ps = outp.tile([P, CHUNK], F32, tag="out", name=f"p3ps{m}_{acc}")
                    for i, g in enumerate(heads):
                        nc.tensor.matmul(ps[:], wo_sb[m][:, g * P:(g + 1) * P],
                                         rhs_t[g][:],
                                         start=(i == 0), stop=(i == len(heads) - 1))
                    return ps

                def attn_block(h, b, qcl):
                    qg = b * (S // CHUNK) + qcl
                    q0 = qg * CHUNK
                    nkt = (CHUNK // P) * (qcl + 1)
                    out_ps = outp.tile([P, CHUNK], F32, tag="out", name=f"out{h}_{qg}")
                    den_ps = denp.tile([1, CHUNK], F32, tag="den", name=f"den{h}_{qg}")

                    def qk_stage(kt):
                        kcol = b * S + kt * P
                        st_ps = stp.tile([P, CHUNK], F32, tag="st",
                                         name=f"st{h}_{qg}_{kt}")
                        nc.tensor.matmul(st_ps[:],
                                         k_n[h][:, kcol:kcol + P],
                                         q_n[h][:, q0:q0 + CHUNK],
                                         start=True, stop=False)
                        nc.tensor.matmul(st_ps[:],
                                         k_pe[b * DR:(b + 1) * DR, kt * P:(kt + 1) * P],
                                         q_rope[h][b * DR:(b + 1) * DR,
                                                   qcl * CHUNK:(qcl + 1) * CHUNK],
                                         start=False, stop=True)
                        pt = ptp.tile([P, CHUNK], CD, tag="pt",
                                      name=f"pt{h}_{qg}_{kt}", bufs=4)
                        nc.scalar.activation(pt[:], st_ps[:], AFT.Exp,
                                             bias=0.0, scale=SCALE)
                        if kt >= (CHUNK // P) * qcl:  # diagonal block
                            ptm = ptp.tile([P, CHUNK], CD, tag="ptm",
                                           name=f"ptm{h}_{qg}_{kt}", bufs=4)
                            nc.gpsimd.affine_select(
                                ptm[:], pt[:], pattern=[[1, CHUNK]],
                                base=qcl * CHUNK - kt * P,
                                channel_multiplier=-1,
                                compare_op=mybir.AluOpType.is_ge, fill=0.0)
                            return ptm
                        return pt

                    pend = {}

                    def av_stage(kt, ptf):
                        nc.tensor.matmul(
                            out_ps[:],
                            v_tok[(b * S // P) + kt][:, h * DV:(h + 1) * DV],
                            ptf[:],
                            start=(kt == 0), stop=(kt == nkt - 1))
                        # 4-way tree-summed denominator
                        r = kt % 4
                        if r in (0, 2):
                            pend[r] = ptf
                        elif r == 1:
                            pd = ptp.tile([P, CHUNK], CD, tag="pden",
                                          name=f"pden{h}_{qg}_{kt}", bufs=2)
                            nc.vector.tensor_add(pd[:], pend[0][:], ptf[:])
                            pend[1] = pd
                        else:
                            pd2 = ptp.tile([P, CHUNK], CD, tag="pden",
                                           name=f"pden{h}_{qg}_{kt}", bufs=2)
                            nc.vector.tensor_add(pd2[:], pend[2][:], ptf[:])
                            pd4 = ptp.tile([P, CHUNK], CD, tag="pden4",
                                           name=f"pden4_{h}_{qg}_{kt}", bufs=2)
                            nc.vector.tensor_add(pd4[:], pend[1][:], pd2[:])
                            nc.tensor.matmul(den_ps[:], ones_col[:], pd4[:],
                                             start=(kt == 3),
                                             stop=(kt == nkt - 1))

                    pipe = []
                    for kt in range(nkt):
                        pipe.append(qk_stage(kt))
                        if len(pipe) > 2:
                            av_stage(kt - 2, pipe.pop(0))
                    for j, ptf in enumerate(pipe):
                        av_stage(nkt - len(pipe) + j, ptf)
                    attn = finp.tile([P, CHUNK], CD, tag="attn", bufs=2)
                    nc.vector.tensor_copy(attn[:], out_ps[:])
                    den_c = finp.tile([1, CHUNK], CD, tag="denc", bufs=2)
                    nc.scalar.copy(den_c[:], den_ps[:])
                    nc.sync.dma_start(
                        a2a_in[h][qg * (DV + 1):qg * (DV + 1) + DV, :], attn[:])
                    nc.sync.dma_start(
                        a2a_in[h][qg * (DV + 1) + DV:(qg + 1) * (DV + 1), :],
                        den_c[:])

                # ---- h0: attention, then its A2A
                for b in range(B):
                    for qcl in range(S // CHUNK):
                        attn_block(0, b, qcl)
                nc.gpsimd.collective_compute(
                    "AllToAll", mybir.AluOpType.bypass, replica_groups=groups,
                    ins=[a2a_in[0].ap().opt()], outs=[a2a_out[0].ap().opt()])

                # ---- h1: attention, then its A2A
                for b in range(B):
                    for qcl in range(S // CHUNK):
                        attn_block(1, b, qcl)
                nc.gpsimd.collective_compute(
                    "AllToAll", mybir.AluOpType.bypass, replica_groups=groups,
                    ins=[a2a_in[1].ap().opt()], outs=[a2a_out[1].ap().opt()])

                # ---- phase-3 even half: runs during the h1 A2A flight
                for g in order:
                    load_head(g)
                for m in range(DIM // P):
                    ps = partial_group(m, order, 0)
                    pa = rp3.tile([P, CHUNK], CD, tag=f"pa{m}", name=f"pa{m}")
                    nc.scalar.copy(pa[:], ps[:])
                    partials[m] = pa

                # ---- phase-3 odd half (tail)
                for g in order2:
                    load_head(g)
                with tc.tile_pool(name="p3o", bufs=2) as op3:
                    for m in range(DIM // P):
                        ps = partial_group(m, order2, 1)
                        ot = op3.tile([P, CHUNK], F32, tag="ot")
                        nc.vector.tensor_add(ot[:], ps[:], partials[m][:])
                        nc.sync.dma_start(outT[m * P:(m + 1) * P, :], ot[:])
            _rp3cm.__exit__(None, None, None)

        _wp3cm.__exit__(None, None, None)
        _nrmcm.__exit__(None, None, None)
        _akvcm.__exit__(None, None, None)
        _wp2cm.__exit__(None, None, None)
        _vp2cm.__exit__(None, None, None)
        _kp2cm.__exit__(None, None, None)
        _qp2cm.__exit__(None, None, None)

    nc.compile()
    return nc


def _tile_kxm(w, nk, nm):
    """(nk*128, nm*128) -> (nm, 128, nk*128): [m][p][kt*128+j] = w[kt*128+p, m*128+j]."""
    return np.ascontiguousarray(
        w.reshape(nk, P, nm, P).transpose(2, 1, 0, 3).reshape(nm, P, nk * P))


_CACHE = {}


def _prep(inputs):
    x = np.asarray(inputs["x"], np.float32)
    fc = np.asarray(inputs["freqs_cos"], np.float32)
    fs = np.asarray(inputs["freqs_sin"], np.float32)
    wq_a = np.asarray(inputs["wq_a"], np.float32)
    q_norm_w = np.asarray(inputs["q_norm_w"], np.float32)
    wq_b = np.asarray(inputs["wq_b"], np.float32)
    wkv_a = np.asarray(inputs["wkv_a"], np.float32)
    kv_norm_w = np.asarray(inputs["kv_norm_w"], np.float32)
    wkv_b = np.asarray(inputs["wkv_b"], np.float32)
    wo = np.asarray(inputs["wo"], np.float32)

    x_flat = x.reshape(T, DIM)

    wqaT_t = _tile_kxm(wq_a.T, DIM // P, QL // P)

    at = wkv_a.T                                     # (DIM, 576)
    Rw = at[:, KVL::2]
    Iw = at[:, KVL + 1::2]
    wkvaT_t = _tile_kxm(np.concatenate([at[:, :KVL], Rw, Iw, Iw, Rw], axis=1),
                        DIM // P, KVE // P)

    wqb_sT = (wq_b * q_norm_w[None, :]).T            # (QL, H*192)
    wkvb_sT = (wkv_b * kv_norm_w[None, :]).T         # (KVL, H*256)

    woT_t = _tile_kxm(wo.T, DIM // P, DIM // P)

    cT, sT = fc.T, fs.T
    cosbM = np.concatenate([cT, cT, cT, cT], axis=0)   # (128, S)
    sinbM = np.concatenate([-sT, sT, -sT, sT], axis=0)
    permM = np.zeros((P, P), np.float32)
    permM[np.arange(P) ^ 32, np.arange(P)] = 1.0

    # wqb for ALL heads: 16 nope m-tiles then 8 rope m-tiles (head pairs,
    # de-interleaved [R_2j | I_2j | R_2j+1 | I_2j+1])
    qb = [wqb_sT[:, h * 192: h * 192 + DN] for h in range(H)]
    for j in range(H // 2):
        for h in (2 * j, 2 * j + 1):
            rope = wqb_sT[:, h * 192 + DN:(h + 1) * 192]
            qb.append(rope[:, 0::2])
            qb.append(rope[:, 1::2])
    wqbT_t = _tile_kxm(np.concatenate(qb, axis=1), QL // P, NM_Q)

    in_maps = []
    for c in range(NC):
        h0, h1 = 2 * c, 2 * c + 1
        kn = [wkvb_sT[:, h * 256: h * 256 + DN] for h in (h0, h1)]
        vv = [wkvb_sT[:, h * 256 + DN: (h + 1) * 256] for h in (h0, h1)]
        wkvbTk_c = _tile_kxm(np.concatenate(kn, axis=1), KVL // P, 2)
        wkvbTv_c = np.ascontiguousarray(np.concatenate(vv, axis=1))

        pos0 = (c * TPC) % S
        in_maps.append({
            "xT": np.ascontiguousarray(x_flat[c * TPC:(c + 1) * TPC].T).astype(BF),
            "wqaT": wqaT_t.astype(BF), "wkvaT": wkvaT_t.astype(BF),
            "wqbT": wqbT_t.astype(BF), "wkvbTk": wkvbTk_c.astype(BF),
            "wkvbTv": wkvbTv_c.astype(BF),
            "woT": woT_t.astype(BF), "perm": permM.astype(BF),
            "cosc": np.ascontiguousarray(cosbM[:, pos0:pos0 + TPC]).astype(BF),
            "sinc": np.ascontiguousarray(sinbM[:, pos0:pos0 + TPC]).astype(BF),
        })
    return in_maps


def kernel(**inputs):
    in_maps = _prep(inputs)
    if "nc" not in _CACHE:
        _CACHE["nc"] = build_nc()
    r = run_bass_kernel_spmd(_CACHE["nc"], in_maps, list(range(NC)))
    out_flat = np.empty((T, DIM), np.float32)
    for c in range(NC):
        out_flat[c * TPC:(c + 1) * TPC] = r.results[c]["outT"].T
    return out_flat.reshape(B, S, DIM)
